# revision 1
# baseline (speedup 1.0000x reference)
"""Trainium2 Bass kernel for single-head causal attention with RoPE.

Problem: B=4, S=4096, D=2048, H=1.
  out = softmax(causal(rope(q@Wq) @ rope(q@Wk)^T / sqrt(D))) @ (q@Wv) @ Wo

Sharding: 8 cores = 4 batches x 2 groups. Each core owns 4 of the batch's 8
512-row blocks ({7,4,3,0} even cores / {6,5,2,1} odd) — a causal-balanced
split: both cores see 18 real key-steps, padded to a uniform per-slot
schedule TSTEPS=[8,6,4,2] so all cores run one NEFF. K/V projections are
computed only for OWN blocks; the pair exchanges K/V per block through
2-rank AllGathers that overlap the projections.

All matmuls bf16 (PSUM accumulates fp32). RoPE is reduced to half-rotation by
permuting Wq/Wk columns on the host (interleaved pairs -> halves); the score
scale 1/sqrt(D) is folded into Wq. Attention is transpose-free: scores are
computed as S^T[k,q] = K^T.T @ Q^T per block, softmax runs without
max-subtraction (scores are O(+-10) for this data), the denominator is
accumulated on the vector engine and reduced across partitions with a single
fp32 ones-matmul per slot, and P^T feeds the PV matmul directly.

Schedule notes (v2): K and V projections are fused per key block (one q-tile
load feeds both), input DMAs are spread across the sync/scalar/vector/gpsimd
queues so compute starts ~5us in and phase transitions stay fed, the last Q
projection slot stays in SBUF for the first attention slot, the first
attention K/V tiles prefetch during the Q projection, and the fp32->bf16
output conversion is fused into the last PV accumulation.
"""

import json
import math
import os

import ml_dtypes
import numpy as np

import concourse.bass as bass
import concourse.mybir as mybir
import concourse.tile as tile
from concourse.bass_utils import run_bass_kernel_spmd


def _split_multi_waits(bir_json_bytes):
    """Rewrite BIR so no instruction carries more than one semaphore wait.

    The walrus build in this environment rejects instructions with >1 sync
    wait. Extra waits are hoisted onto injected same-engine EventSemaphore
    instructions placed immediately before the instruction (engine program
    order makes them gate it)."""
    d = json.loads(bir_json_bytes)
    for fn in d["functions"]:
        for blk in fn["blocks"]:
            out = []
            for inst in blk["instructions"]:
                si = inst.get("sync_info") or {}
                ow = si.get("on_wait") or []
                if len(ow) > 1:
                    for i, w in enumerate(ow[:-1]):
                        out.append({
                            "debug": inst.get("debug"),
                            "engine": inst["engine"],
                            "ins": [],
                            "outs": [],
                            "name": f"{inst['name']}_sw{i}",
                            "opcode": "EventSemaphore",
                            "sync_info": {"on_update": [], "on_wait": [w]},
                        })
                    si["on_wait"] = [ow[-1]]
                out.append(inst)
            blk["instructions"] = out
    return json.dumps(d).encode()


def _install_split_waits():
    import concourse.bass_utils as bu
    if getattr(bu, "_split_waits_installed", False):
        return
    orig = bu.compile_bir_kernel

    def patched(bir_json, tmpdir, neff_name="file.neff"):
        return orig(_split_multi_waits(bir_json), tmpdir, neff_name)

    bu.compile_bir_kernel = patched
    bu._split_waits_installed = True
    import concourse.bass2jax as b2j
    if getattr(b2j, "compile_bir_kernel", None) is orig:
        b2j.compile_bir_kernel = patched


_install_split_waits()

BF = mybir.dt.bfloat16
F32 = mybir.dt.float32
bf16 = ml_dtypes.bfloat16

B, S, D = 4, 4096, 2048
HALF = D // 2
P = 128
QB = 512           # query block (one slot)
KB = 512           # key step
NSLOT = 4          # query blocks per core
NQ = NSLOT * QB    # 2048 own queries per core
DI = D // P        # 16 contraction chunks
NCH = D // P       # 16 output chunks
TSTEPS = [8, 6, 4, 2]   # padded key steps per slot (uniform across cores)
BLOCKS_EVEN = [7, 4, 3, 0]
BLOCKS_ODD = [6, 5, 2, 1]
KEYS_EVEN = [0, 3, 4, 7]   # same blocks, ascending (AllGather pairing order)
KEYS_ODD = [1, 2, 5, 6]
# key block s of the batch lives at gathered[AGIDX[s][0]][AGIDX[s][1]]
AGIDX = [(0, 0), (0, 1), (1, 1), (1, 0), (2, 0), (2, 1), (3, 1), (3, 0)]
KTSZ = D * KB              # elements of one K^T block [D, 512]
ROPE_BASE = 10000.0
NEG = -1.0e30


def _dma_in(nc, dst, src_ap, n, eng=None):
    """Per-chunk DMA load of a [P, n, F] tile from a "(c p) f" DRAM view."""
    v = src_ap.rearrange("(c p) f -> p c f", p=P)
    e = eng if eng is not None else nc.sync
    for c in range(n):
        e.dma_start(dst[:, c], v[:, c])


def _dma_out(nc, dst_ap, src, n, eng=None):
    """Per-chunk DMA store of a [P, n, F] tile to a "(c p) f" DRAM view."""
    v = dst_ap.rearrange("(c p) f -> p c f", p=P)
    e = eng if eng is not None else nc.sync
    for c in range(n):
        e.dma_start(v[:, c], src[:, c])


def _build():
    nc = bass.Bass(num_devices=8)

    qT_own = nc.declare_dram_parameter("qT_own", [D, NQ], BF, isOutput=False)
    qT_keys = nc.declare_dram_parameter("qT_keys", [D, NQ], BF, isOutput=False)
    Wq = nc.declare_dram_parameter("Wq", [D, D], BF, isOutput=False)
    Wk = nc.declare_dram_parameter("Wk", [D, D], BF, isOutput=False)
    Wv = nc.declare_dram_parameter("Wv", [D, D], BF, isOutput=False)
    Wo = nc.declare_dram_parameter("Wo", [D, D], BF, isOutput=False)
    cosO = nc.declare_dram_parameter("cosO", [HALF, NQ], BF, isOutput=False)
    sinO = nc.declare_dram_parameter("sinO", [HALF, NQ], BF, isOutput=False)
    cosK = nc.declare_dram_parameter("cosK", [HALF, NQ], BF, isOutput=False)
    sinK = nc.declare_dram_parameter("sinK", [HALF, NQ], BF, isOutput=False)
    masks = nc.declare_dram_parameter("masks", [NSLOT, 2, KB, QB], BF,
                                      isOutput=False)
    out = nc.declare_dram_parameter("out", [NQ, D], F32, isOutput=True)

    from contextlib import ExitStack
    with tile.TileContext(nc) as tc:
        with ExitStack() as top:
            dram = top.enter_context(
                tc.tile_pool(name="dram", bufs=1, space="DRAM"))
            QT_d = dram.tile([D, NQ], BF, tag="QT_d")   # slot-3 cols unused
            l_d = dram.tile([NSLOT, QB], F32, tag="l_d")
            kvK = [dram.tile([KTSZ], BF, tag=f"kvK{i}", name=f"kvK{i}")
                   for i in range(4)]
            kvV = [dram.tile([KB * D], BF, tag=f"kvV{i}", name=f"kvV{i}")
                   for i in range(4)]
            gK = [dram.tile([2, KTSZ], BF, tag=f"gK{i}", name=f"gK{i}")
                  for i in range(4)]
            gV = [dram.tile([2, KB * D], BF, tag=f"gV{i}", name=f"gV{i}")
                  for i in range(4)]

            def _ag(src_t, dst_t):
                nc.gpsimd.collective_compute(
                    "AllGather",
                    mybir.AluOpType.bypass,
                    replica_groups=[[0, 1], [2, 3], [4, 5], [6, 7]],
                    ins=[src_t[:].opt()],
                    outs=[dst_t[:].opt()],
                )

            def _rope_block(w_t, q_t, cs, ko_ap, pps, tmp,
                            post_j=None):
                """One 512-column projection block with half-RoPE epilogue.

                cs = (cosA, cosB, sinA, sinB) quarter-split tiles so the
                WAR on each frees mid-block. ko_ap(j) -> (ap_lo, ap_hi):
                destination APs for output chunks j and j+8. post_j runs
                after the epilogue (e.g. to store the chunks)."""
                cosA, cosB, sinA, sinB = cs
                for j in range(8):
                    psA = pps.tile([P, 512], F32, tag="psA")
                    psB = pps.tile([P, 512], F32, tag="psB")
                    for di in range(DI):
                        nc.tensor.matmul(
                            psA, w_t[:, di, j * P:(j + 1) * P],
                            q_t[:, di, :],
                            start=(di == 0), stop=(di == DI - 1))
                    for di in range(DI):
                        nc.tensor.matmul(
                            psB, w_t[:, di, (j + 8) * P:(j + 9) * P],
                            q_t[:, di, :],
                            start=(di == 0), stop=(di == DI - 1))
                    ap_lo, ap_hi = ko_ap(j)
                    cj = cosA[:, j] if j < 4 else cosB[:, j - 4]
                    sj = sinA[:, j] if j < 4 else sinB[:, j - 4]
                    t1 = tmp.tile([P, 512], BF, tag="t1")
                    t2 = tmp.tile([P, 512], BF, tag="t2")
                    nc.vector.tensor_tensor(
                        t1, psA, cj, mybir.AluOpType.mult)
                    nc.vector.tensor_tensor(
                        t2, psB, sj, mybir.AluOpType.mult)
                    nc.vector.tensor_tensor(
                        ap_lo, t1, t2, mybir.AluOpType.subtract)
                    nc.vector.tensor_tensor(
                        t1, psA, sj, mybir.AluOpType.mult)
                    nc.vector.tensor_tensor(
                        t2, psB, cj, mybir.AluOpType.mult)
                    nc.vector.tensor_tensor(
                        ap_hi, t1, t2, mybir.AluOpType.add)
                    if post_j is not None:
                        post_j(j, ap_lo, ap_hi)

            # q3p sits at the bottom of the allocation stack: it carries the
            # last Q-projection slot (attention's first slot) across the
            # phase boundary, avoiding a serialized DRAM roundtrip there.
            q3p = top.enter_context(tc.tile_pool(name="q3p", bufs=1))
            q3_t = q3p.tile([P, DI, QB], BF, tag="q3")

            # --------- unified projection phase: K+V fused, then Q ---------
            with ExitStack() as pstk:
                qio = pstk.enter_context(tc.tile_pool(name="qio", bufs=2))
                csio = pstk.enter_context(tc.tile_pool(name="csio", bufs=1))
                kcp = pstk.enter_context(tc.tile_pool(name="kc", bufs=6))
                vop = pstk.enter_context(tc.tile_pool(name="vo", bufs=1))
                tmp = pstk.enter_context(tc.tile_pool(name="tmp", bufs=1))
                pps = pstk.enter_context(
                    tc.tile_pool(name="pps", bufs=2, space="PSUM"))
                vps = pstk.enter_context(
                    tc.tile_pool(name="vps", bufs=4, space="PSUM"))
                # weight pools on top of the stack so Wk's region can be
                # popped and re-pushed as Wq mid-phase
                w2_cm = tc.tile_pool(name="wv", bufs=1)
                w2p = w2_cm.__enter__()
                w1_cm = tc.tile_pool(name="wk", bufs=1)
                w1p = w1_cm.__enter__()

                wk_t = w1p.tile([P, DI, D], BF, tag="WK")
                wv_t = w2p.tile([P, DI, D], BF, tag="WV")
                wq_t = None
                for kb in range(4):
                    sl = slice(kb * 512, (kb + 1) * 512)
                    q_t = qio.tile([P, DI, 512], BF, tag="qin")
                    cs = (csio.tile([P, 4, 512], BF, tag="cosA", name="cosA"),
                          csio.tile([P, 4, 512], BF, tag="cosB", name="cosB"),
                          csio.tile([P, 4, 512], BF, tag="sinA", name="sinA"),
                          csio.tile([P, 4, 512], BF, tag="sinB", name="sinB"))
                    if kb == 0:
                        # startup: split q0+Wk chunk-interleaved across the
                        # sync and scalar queues; cos/sin go to gpsimd
                        qv = qT_keys[:, sl].rearrange("(c p) f -> p c f", p=P)
                        wkv = Wk[:, :].rearrange("(c p) f -> p c f", p=P)
                        for c in range(DI):
                            e = nc.sync if c % 2 == 0 else nc.scalar
                            e.dma_start(q_t[:, c], qv[:, c])
                            e.dma_start(wk_t[:, c], wkv[:, c])
                        _dma_in(nc, cs[0], cosK[0:HALF // 2, sl], 4,
                                eng=nc.gpsimd)
                        _dma_in(nc, cs[1], cosK[HALF // 2:HALF, sl], 4,
                                eng=nc.gpsimd)
                        _dma_in(nc, cs[2], sinK[0:HALF // 2, sl], 4,
                                eng=nc.gpsimd)
                        _dma_in(nc, cs[3], sinK[HALF // 2:HALF, sl], 4,
                                eng=nc.gpsimd)
                    else:
                        _dma_in(nc, q_t, qT_keys[:, sl], DI, eng=nc.sync)
                        _dma_in(nc, cs[0], cosK[0:HALF // 2, sl], 4,
                                eng=nc.sync)
                        _dma_in(nc, cs[1], cosK[HALF // 2:HALF, sl], 4,
                                eng=nc.sync)
                        _dma_in(nc, cs[2], sinK[0:HALF // 2, sl], 4,
                                eng=nc.sync)
                        _dma_in(nc, cs[3], sinK[HALF // 2:HALF, sl], 4,
                                eng=nc.sync)
                    kv_out = kvK[kb][:].rearrange(
                        "(d s) -> d s", s=KB).rearrange(
                        "(c p) f -> p c f", p=P)

                    def _koap(j, kcp=kcp):
                        lo = kcp.tile([P, 512], BF, tag="koc")
                        hi = kcp.tile([P, 512], BF, tag="koc")
                        return lo, hi

                    def _kstore(j, lo, hi, kv_out=kv_out, kb=kb):
                        nc.gpsimd.dma_start(kv_out[:, j], lo)
                        nc.gpsimd.dma_start(kv_out[:, j + 8], hi)
                        if kb == 0 and j == 0:
                            _dma_in(nc, wv_t, Wv, DI, eng=nc.gpsimd)

                    _rope_block(wk_t, q_t, cs, _koap, pps, tmp,
                                post_j=_kstore)
                    if kb == 3:
                        # free Wk's region and stream Wq into it, so it
                        # lands during block 3's V part
                        w1_cm.__exit__(None, None, None)
                        defer_ag = True
                        w1_cm = tc.tile_pool(name="wq", bufs=1)
                        w1p = w1_cm.__enter__()
                        wq_t = w1p.tile([P, DI, D], BF, tag="WQ")
                        wqv = Wq[:, :].rearrange("(c p) f -> p c f", p=P)
                        for c in range(DI):
                            e = nc.sync if c % 2 == 0 else nc.gpsimd
                            e.dma_start(wq_t[:, c], wqv[:, c])
                    if kb != 3:
                        _ag(kvK[kb], gK[kb])
                    if kb == 3:
                        # hoist Q slot 0's inputs so they stream during
                        # block 3's V part (their pool WARs are already
                        # resolved; only the sync queue order held them)
                        q_sb0 = qio.tile([P, DI, 512], BF, tag="qin")
                        _dma_in(nc, q_sb0, qT_own[:, 0:512], DI, eng=nc.sync)
                        cs_sb0 = (
                            csio.tile([P, 4, 512], BF, tag="cosA",
                                      name="cosA0"),
                            csio.tile([P, 4, 512], BF, tag="cosB",
                                      name="cosB0"),
                            csio.tile([P, 4, 512], BF, tag="sinA",
                                      name="sinA0"),
                            csio.tile([P, 4, 512], BF, tag="sinB",
                                      name="sinB0"))
                        _dma_in(nc, cs_sb0[0], cosO[0:HALF // 2, 0:512], 4,
                                eng=nc.sync)
                        _dma_in(nc, cs_sb0[1], cosO[HALF // 2:HALF, 0:512],
                                4, eng=nc.sync)
                        _dma_in(nc, cs_sb0[2], sinO[0:HALF // 2, 0:512], 4,
                                eng=nc.sync)
                        _dma_in(nc, cs_sb0[3], sinO[HALF // 2:HALF, 0:512],
                                4, eng=nc.sync)
                    # V part reuses the same q tile
                    vv = kvV[kb][:].rearrange("(s d) -> s d", d=D)
                    for ss in range(4):
                        vo = vop.tile([P, D], BF, tag="vo")
                        for dob in range(4):
                            ps = vps.tile([P, 512], F32, tag="vps")
                            for di in range(DI):
                                nc.tensor.matmul(
                                    ps, q_t[:, di, ss * P:(ss + 1) * P],
                                    wv_t[:, di, dob * 512:(dob + 1) * 512],
                                    start=(di == 0), stop=(di == DI - 1))
                            nc.scalar.copy(
                                vo[:, dob * 512:(dob + 1) * 512], ps)
                        nc.scalar.dma_start(vv[ss * P:(ss + 1) * P, :], vo)
                    if kb != 3:
                        _ag(kvV[kb], gV[kb])

                # ---------------- Q projection (own query slots) ----------
                # Slots 0-2 stream to DRAM chunk-by-chunk; slot 3 writes
                # into the resident q3 tile.
                for sb in range(NSLOT):
                    sl = slice(sb * 512, (sb + 1) * 512)
                    if sb == 0:
                        q_t = q_sb0
                        cs = cs_sb0
                    else:
                        q_t = qio.tile([P, DI, 512], BF, tag="qin")
                        _dma_in(nc, q_t, qT_own[:, sl], DI, eng=nc.sync)
                        cs = (csio.tile([P, 4, 512], BF, tag="cosA",
                                        name="cosA"),
                              csio.tile([P, 4, 512], BF, tag="cosB",
                                        name="cosB"),
                              csio.tile([P, 4, 512], BF, tag="sinA",
                                        name="sinA"),
                              csio.tile([P, 4, 512], BF, tag="sinB",
                                        name="sinB"))
                        _dma_in(nc, cs[0], cosO[0:HALF // 2, sl], 4,
                                eng=nc.sync)
                        _dma_in(nc, cs[1], cosO[HALF // 2:HALF, sl], 4,
                                eng=nc.sync)
                        _dma_in(nc, cs[2], sinO[0:HALF // 2, sl], 4,
                                eng=nc.sync)
                        _dma_in(nc, cs[3], sinO[HALF // 2:HALF, sl], 4,
                                eng=nc.sync)
                    if sb == 3:
                        def _koap_q(j):
                            return q3_t[:, j], q3_t[:, j + 8]
                        _rope_block(wq_t, q_t, cs, _koap_q,
                                    pps, tmp)
                    else:
                        qd_out = QT_d[:, sb * QB:(sb + 1) * QB].rearrange(
                            "(c p) f -> p c f", p=P)

                        def _koap_q(j, kcp=kcp):
                            lo = kcp.tile([P, 512], BF, tag="koc")
                            hi = kcp.tile([P, 512], BF, tag="koc")
                            return lo, hi

                        def _qstore(j, lo, hi, qd_out=qd_out):
                            nc.gpsimd.dma_start(qd_out[:, j], lo)
                            nc.gpsimd.dma_start(qd_out[:, j + 8], hi)

                        _rope_block(wq_t, q_t, cs, _koap_q,
                                    pps, tmp, post_j=_qstore)
                _ag(kvK[3], gK[3])
                _ag(kvV[3], gV[3])
                w1_cm.__exit__(None, None, None)   # wq
                w2_cm.__exit__(None, None, None)   # wv

            # ------------- attention + output projection -------------
            with ExitStack() as stk:
                qslot = stk.enter_context(tc.tile_pool(name="qslot", bufs=2))
                kio = stk.enter_context(tc.tile_pool(name="kio", bufs=2))
                vio = stk.enter_context(tc.tile_pool(name="vio", bufs=2))
                ptpool = stk.enter_context(tc.tile_pool(name="pt", bufs=2))
                mio = stk.enter_context(tc.tile_pool(name="mio", bufs=2))
                otpool = stk.enter_context(tc.tile_pool(name="ot", bufs=1))
                accpool = stk.enter_context(tc.tile_pool(name="acc", bufs=1))
                wopool = stk.enter_context(tc.tile_pool(name="wo", bufs=2))
                fopool = stk.enter_context(tc.tile_pool(name="fo", bufs=2))
                small = stk.enter_context(tc.tile_pool(name="small", bufs=1))
                stps = stk.enter_context(
                    tc.tile_pool(name="stps", bufs=2, space="PSUM"))
                pvps = stk.enter_context(
                    tc.tile_pool(name="pvps", bufs=3, space="PSUM"))
                lps = stk.enter_context(
                    tc.tile_pool(name="lps", bufs=1, space="PSUM"))
                fps = stk.enter_context(
                    tc.tile_pool(name="fps", bufs=2, space="PSUM"))
                ones_t = small.tile([P, 1], F32, tag="ones")
                nc.vector.memset(ones_t, 1.0)
                ones_bf = small.tile([P, 1], BF, tag="onesb")
                nc.vector.memset(ones_bf, 1.0)
                q_next = {}
                for j in reversed(range(NSLOT)):   # ascending step counts
                    t = TSTEPS[j]
                    q_t = q3_t if j == 3 else q_next.pop(j)
                    kt0 = None
                    if j == 3:
                        # first K tile split across sync+scalar queues so
                        # the first score chain starts ~5us after the
                        # projection phase drains
                        kt0 = kio.tile([P, DI, KB], BF, tag="kt")
                        kv0 = gK[0][0].rearrange(
                            "(d s) -> d s", s=KB).rearrange(
                            "(c p) f -> p c f", p=P)
                        for c in range(DI):
                            e = nc.sync if c % 2 == 0 else nc.scalar
                            e.dma_start(kt0[:, c], kv0[:, c])
                    # prefetch both boundary masks for this slot
                    m_ts = []
                    for sidx in range(2):
                        m_t = mio.tile([P, 4, QB], BF, tag="mask")
                        _dma_in(nc, m_t, masks[j, sidx], 4, eng=nc.gpsimd)
                        m_ts.append(m_t)
                    # prefetch Wo chunk 0 for this slot's output proj
                    wo0 = wopool.tile([P, DI, 512], BF, tag="wo")
                    _dma_in(nc, wo0, Wo[:, 0:512], DI, eng=nc.gpsimd)

                    ot = otpool.tile([P, NCH, QB], BF, tag="ot")
                    acc = accpool.tile([P, QB], F32, tag="acc")
                    for s in range(t):
                        ag, idx = AGIDX[s]
                        if j == 3 and s == 0:
                            kt = kt0
                        else:
                            kt = kio.tile([P, DI, KB], BF, tag="kt")
                            _dma_in(nc, kt,
                                    gK[ag][idx].rearrange("(d s) -> d s",
                                                          s=KB),
                                    DI, eng=nc.sync)
                        vt = vio.tile([P, 4, D], BF, tag="vt")
                        _dma_in(nc, vt,
                                gV[ag][idx].rearrange("(s d) -> s d", d=D),
                                4, eng=nc.gpsimd if j == 3 else nc.sync)
                        pt = ptpool.tile([P, 4, QB], BF, tag="pt")
                        masked = s >= t - 2
                        for kc in range(4):
                            st = stps.tile([P, QB], F32, tag="st")
                            for di in range(DI):
                                nc.tensor.matmul(
                                    st, kt[:, di, kc * P:(kc + 1) * P],
                                    q_t[:, di, :],
                                    start=(di == 0), stop=(di == DI - 1))
                            if masked:
                                nc.vector.tensor_add(
                                    st, st, m_ts[s - (t - 2)][:, kc])
                            nc.scalar.activation(
                                pt[:, kc], st,
                                mybir.ActivationFunctionType.Exp)
                            # softmax denominator: accumulate on gpsimd up
                            # to step t-2, reduce it early with the fp32
                            # ones-matmul, then fold the last step's pt in
                            # with tiny bf16 ones-matmuls so l is ready
                            # before the output projection needs it
                            if s < t - 1:
                                if s == 0 and kc == 0:
                                    nc.gpsimd.tensor_copy(acc, pt[:, kc])
                                else:
                                    nc.gpsimd.tensor_tensor(
                                        acc, acc, pt[:, kc],
                                        mybir.AluOpType.add)
                        if s == t - 2:
                            l_ps = lps.tile([1, QB], F32, tag="lps")
                            nc.tensor.matmul(l_ps, ones_t, acc,
                                             start=True, stop=False)
                        elif s == t - 1:
                            for kc in range(4):
                                nc.tensor.matmul(
                                    l_ps, ones_bf, pt[:, kc],
                                    start=False, stop=(kc == 3))
                            # resolve the denominator roundtrip during this
                            # step's PV so inv_l is ready for the O proj
                            l_sb = small.tile([1, QB], F32, tag="lsb")
                            nc.scalar.copy(l_sb, l_ps)
                            nc.sync.dma_start(l_d[j:j + 1, :], l_sb)
                            lcols = small.tile([P, NSLOT], F32, tag="lcols")
                            nc.sync.dma_start(
                                lcols, l_d[j].rearrange("(qs p) -> p qs",
                                                        p=P))
                            inv_l = small.tile([P, NSLOT], F32, tag="invl")
                            nc.vector.reciprocal(inv_l, lcols)
                        for do in range(NCH):
                            pv = pvps.tile([P, QB], F32, tag="pv")
                            for kc in range(4):
                                nc.tensor.matmul(
                                    pv, vt[:, kc, do * P:(do + 1) * P],
                                    pt[:, kc, :],
                                    start=(kc == 0), stop=(kc == 3))
                            if s == 0:
                                nc.scalar.copy(ot[:, do], pv)
                            else:
                                nc.vector.tensor_add(ot[:, do],
                                                     ot[:, do], pv)
                    if j > 0:
                        nt = qslot.tile([P, DI, QB], BF, tag="qslot")
                        _dma_in(nc, nt,
                                QT_d[:, (j - 1) * QB:j * QB], DI,
                                eng=nc.scalar)
                        q_next[j - 1] = nt
                    # O projection: fo scaling runs on the scalar engine so
                    # the vector engine never gates the PE here
                    for dob in range(4):
                        if dob == 0:
                            wo_t = wo0
                        else:
                            wo_t = wopool.tile([P, DI, 512], BF, tag="wo")
                            _dma_in(nc, wo_t,
                                    Wo[:, dob * 512:(dob + 1) * 512],
                                    DI, eng=nc.gpsimd)
                        for qs in range(4):
                            f_ps = fps.tile([P, 512], F32, tag="fps")
                            for di in range(DI):
                                nc.tensor.matmul(
                                    f_ps, ot[:, di, qs * P:(qs + 1) * P],
                                    wo_t[:, di, :],
                                    start=(di == 0), stop=(di == DI - 1))
                            fo = fopool.tile([P, 512], F32, tag="fo")
                            nc.scalar.mul(fo, f_ps, inv_l[:, qs:qs + 1])
                            nc.scalar.dma_start(
                                out[j * QB + qs * P:
                                    j * QB + (qs + 1) * P,
                                    dob * 512:(dob + 1) * 512], fo)
    return nc


_NC_CACHE = None


def _get_nc():
    global _NC_CACHE
    if _NC_CACHE is None:
        _NC_CACHE = _build()
    return _NC_CACHE


def _host_prep(q, W_q, W_k, W_v, W_o):
    perm = np.concatenate([np.arange(0, D, 2), np.arange(1, D, 2)])
    scale = 1.0 / math.sqrt(D)
    Wq_p = np.ascontiguousarray((W_q * scale)[:, perm]).astype(bf16)
    Wk_p = np.ascontiguousarray(W_k[:, perm]).astype(bf16)
    Wv_p = W_v.astype(bf16)
    Wo_p = W_o.astype(bf16)
    inv_freq = 1.0 / (ROPE_BASE ** (np.arange(0, D, 2, dtype=np.float64) / D))
    ang = np.arange(S, dtype=np.float64)[:, None] * inv_freq[None, :]
    cosT = np.ascontiguousarray(np.cos(ang).T).astype(bf16)   # (HALF, S)
    sinT = np.ascontiguousarray(np.sin(ang).T).astype(bf16)
    return Wq_p, Wk_p, Wv_p, Wo_p, cosT, sinT


def _make_masks(blocks):
    m = np.zeros((NSLOT, 2, KB, QB), dtype=np.float32)
    k_idx = np.arange(KB)[:, None]
    q_idx = np.arange(QB)[None, :]
    tri = np.where(k_idx <= q_idx, 0.0, NEG).astype(np.float32)
    for j, blk in enumerate(blocks):
        t = TSTEPS[j]
        limit = blk + 1
        for sidx, s in enumerate([t - 2, t - 1]):
            if s == limit - 1:
                m[j, sidx] = tri
            elif s >= limit:
                m[j, sidx] = NEG
    return m.astype(bf16)


def run(inputs, trace=False):
    q = np.asarray(inputs["q"], dtype=np.float32)
    W_q = np.asarray(inputs["W_q"], dtype=np.float32)
    W_k = np.asarray(inputs["W_k"], dtype=np.float32)
    W_v = np.asarray(inputs["W_v"], dtype=np.float32)
    W_o = np.asarray(inputs["W_o"], dtype=np.float32)

    Wq_p, Wk_p, Wv_p, Wo_p, cosT, sinT = _host_prep(q, W_q, W_k, W_v, W_o)

    in_maps = []
    core_blocks = []
    for c in range(8):
        b = c // 2
        blocks = BLOCKS_EVEN if c % 2 == 0 else BLOCKS_ODD
        keys = KEYS_EVEN if c % 2 == 0 else KEYS_ODD
        core_blocks.append((b, blocks))
        qTb = np.ascontiguousarray(q[b].T).astype(bf16)       # (D, S)
        own_cols = np.concatenate(
            [np.arange(blk * QB, (blk + 1) * QB) for blk in blocks])
        key_cols = np.concatenate(
            [np.arange(blk * QB, (blk + 1) * QB) for blk in keys])
        in_maps.append({
            "qT_own": np.ascontiguousarray(qTb[:, own_cols]),
            "qT_keys": np.ascontiguousarray(qTb[:, key_cols]),
            "Wq": Wq_p, "Wk": Wk_p, "Wv": Wv_p, "Wo": Wo_p,
            "cosO": np.ascontiguousarray(cosT[:, own_cols]),
            "sinO": np.ascontiguousarray(sinT[:, own_cols]),
            "cosK": np.ascontiguousarray(cosT[:, key_cols]),
            "sinK": np.ascontiguousarray(sinT[:, key_cols]),
            "masks": _make_masks(blocks),
        })

    nc = _get_nc()
    res = run_bass_kernel_spmd(nc, in_maps, core_ids=list(range(8)),
                               trace=trace)

    out = np.zeros((B, S, D), dtype=np.float32)
    for c, (b, blocks) in enumerate(core_blocks):
        o = res.results[c]["out"]
        for j, blk in enumerate(blocks):
            out[b, blk * QB:(blk + 1) * QB] = o[j * QB:(j + 1) * QB]
    return out, res


def kernel(**inputs):
    return run(inputs, trace=False)[0]



# revision 32
# speedup vs baseline: 1.1866x; 1.1866x over previous
"""Trainium2 Bass kernel for single-head causal attention with RoPE.

Problem: B=4, S=4096, D=2048, H=1.
  out = softmax(causal(rope(q@Wq) @ rope(q@Wk)^T / sqrt(D))) @ (q@Wv) @ Wo

Sharding: 8 cores = 4 batches x 2 groups. Query ownership is 256-row
sub-slots paired so both cores run the same schedule with no padded key
steps: pair p owns sub-slots (A, B) with causal lengths (t, t-1) where
t = TSTEPS[p]; steps 0..t-2 run 512 queries wide, the last step runs 256
(A only). K/V projections are computed for 512-row key blocks (even cores
own blocks {0,3,4,7}, odd {1,2,5,6}) and exchanged through 2-rank
AllGathers that overlap the projections.

All matmuls bf16 (PSUM accumulates fp32). RoPE is reduced to half-rotation
by permuting Wq/Wk columns on the host; the score scale 1/sqrt(D) is folded
into Wq. The output projection is folded into the value projection on the
host ((P(qWv))Wo == P(q(WvWo))), so the PV matmul -- with the probability
tile stationary so PSUM comes out in [query, d] orientation -- produces the
final output directly, up to a per-query softmax-denominator scale applied
by the scalar engine on the way out. Scores are computed transpose-free as
S^T[k,q] = K^T.T @ Q^T, softmax runs without max-subtraction, and the
denominator is accumulated on the gpsimd engine and reduced with
ones-matmuls.

Schedule: Wq streams into its own SBUF region during key-block 2 so the
Q projection starts without a weight wait; all 8 AllGathers fire as soon
as their inputs are stored; the first attention K tile is prefetched into
a reserved tile during the Q projection; the last Q pair stays in SBUF
across the phase boundary.
"""

import json
import math

import ml_dtypes
import numpy as np

import concourse.bass as bass
import concourse.mybir as mybir
import concourse.tile as tile
from concourse.bass_utils import run_bass_kernel_spmd


def _split_multi_waits(bir_json_bytes):
    """Rewrite BIR so no instruction carries more than one semaphore wait.

    The walrus build in this environment rejects instructions with >1 sync
    wait. Extra waits are hoisted onto injected same-engine EventSemaphore
    instructions placed immediately before the instruction (engine program
    order makes them gate it)."""
    d = json.loads(bir_json_bytes)
    for fn in d["functions"]:
        for blk in fn["blocks"]:
            out = []
            for inst in blk["instructions"]:
                si = inst.get("sync_info") or {}
                ow = si.get("on_wait") or []
                if len(ow) > 1:
                    for i, w in enumerate(ow[:-1]):
                        out.append({
                            "debug": inst.get("debug"),
                            "engine": inst["engine"],
                            "ins": [],
                            "outs": [],
                            "name": f"{inst['name']}_sw{i}",
                            "opcode": "EventSemaphore",
                            "sync_info": {"on_update": [], "on_wait": [w]},
                        })
                    si["on_wait"] = [ow[-1]]
                out.append(inst)
            blk["instructions"] = out
    return json.dumps(d).encode()


def _install_split_waits():
    import concourse.bass_utils as bu
    if getattr(bu, "_split_waits_installed", False):
        return
    orig = bu.compile_bir_kernel

    def patched(bir_json, tmpdir, neff_name="file.neff"):
        return orig(_split_multi_waits(bir_json), tmpdir, neff_name)

    bu.compile_bir_kernel = patched
    bu._split_waits_installed = True
    import concourse.bass2jax as b2j
    if getattr(b2j, "compile_bir_kernel", None) is orig:
        b2j.compile_bir_kernel = patched


_install_split_waits()

BF = mybir.dt.bfloat16
F32 = mybir.dt.float32
bf16 = ml_dtypes.bfloat16

B, S, D = 4, 4096, 2048
HALF = D // 2
P = 128
QB = 512           # pair width (A+B sub-slots)
QH = 256           # sub-slot width
KB = 512           # key step
NSLOT = 4          # pairs per core
NQ = NSLOT * QB    # 2048 own queries per core
DI = D // P        # 16 contraction chunks
NCH = D // P       # 16 output chunks
TSTEPS = [8, 6, 4, 2]   # key steps per pair (exact, no padding)
# 256-row sub-slot ownership: pair p = (A, B), causal lengths (t, t-1)
PAIRS_EVEN = [(15, 12), (11, 8), (7, 4), (3, 0)]
PAIRS_ODD = [(14, 13), (10, 9), (6, 5), (2, 1)]
KEYS_EVEN = [0, 3, 4, 7]   # K/V 512-block ownership (AllGather pairing)
KEYS_ODD = [1, 2, 5, 6]
# key block s of the batch lives at gathered[AGIDX[s][0]][AGIDX[s][1]]
AGIDX = [(0, 0), (0, 1), (1, 1), (1, 0), (2, 0), (2, 1), (3, 1), (3, 0)]
KTSZ = D * KB              # elements of one K^T block [D, 512]
ROPE_BASE = 10000.0
NEG = -1.0e30


def _dma_in(nc, dst, src_ap, n, eng=None):
    """Per-chunk DMA load of a [P, n, F] tile from a "(c p) f" DRAM view."""
    v = src_ap.rearrange("(c p) f -> p c f", p=P)
    e = eng if eng is not None else nc.sync
    for c in range(n):
        e.dma_start(dst[:, c], v[:, c])


def _dma_out(nc, dst_ap, src, n, eng=None):
    """Per-chunk DMA store of a [P, n, F] tile to a "(c p) f" DRAM view."""
    v = dst_ap.rearrange("(c p) f -> p c f", p=P)
    e = eng if eng is not None else nc.sync
    for c in range(n):
        e.dma_start(v[:, c], src[:, c])


def _build():
    nc = bass.Bass(num_devices=8)

    qT_own = nc.declare_dram_parameter("qT_own", [D, NQ], BF, isOutput=False)
    qT_keys = nc.declare_dram_parameter("qT_keys", [D, NQ], BF, isOutput=False)
    Wq = nc.declare_dram_parameter("Wq", [D, D], BF, isOutput=False)
    Wk = nc.declare_dram_parameter("Wk", [D, D], BF, isOutput=False)
    # Wv here is the host-fused Wv @ Wo, so attention's PV output IS the
    # final output (up to the softmax denominator scale)
    Wv = nc.declare_dram_parameter("Wv", [D, D], BF, isOutput=False)
    cosO = nc.declare_dram_parameter("cosO", [HALF, NQ], BF, isOutput=False)
    sinO = nc.declare_dram_parameter("sinO", [HALF, NQ], BF, isOutput=False)
    cosK = nc.declare_dram_parameter("cosK", [HALF, NQ], BF, isOutput=False)
    sinK = nc.declare_dram_parameter("sinK", [HALF, NQ], BF, isOutput=False)
    masksF = nc.declare_dram_parameter("masksF", [NSLOT, KB, QB], BF,
                                       isOutput=False)
    masksL = nc.declare_dram_parameter("masksL", [NSLOT, KB, QH], BF,
                                       isOutput=False)
    out = nc.declare_dram_parameter("out", [NQ, D], F32, isOutput=True)

    from contextlib import ExitStack
    with tile.TileContext(nc) as tc:
        with ExitStack() as top:
            dram = top.enter_context(
                tc.tile_pool(name="dram", bufs=1, space="DRAM"))
            QT_d = dram.tile([D, NQ], BF, tag="QT_d")   # block-3 cols unused
            l_d = dram.tile([NSLOT, QB], F32, tag="l_d")
            kvK = [dram.tile([KTSZ], BF, tag=f"kvK{i}", name=f"kvK{i}")
                   for i in range(4)]
            kvV = [dram.tile([KB * D], BF, tag=f"kvV{i}", name=f"kvV{i}")
                   for i in range(4)]
            gK = [dram.tile([2, KTSZ], BF, tag=f"gK{i}", name=f"gK{i}")
                  for i in range(4)]
            gV = [dram.tile([2, KB * D], BF, tag=f"gV{i}", name=f"gV{i}")
                  for i in range(4)]

            def _ag(src_t, dst_t):
                nc.gpsimd.collective_compute(
                    "AllGather",
                    mybir.AluOpType.bypass,
                    replica_groups=[[0, 1], [2, 3], [4, 5], [6, 7]],
                    ins=[src_t[:].opt()],
                    outs=[dst_t[:].opt()],
                )

            def _rope_block(w_t, q_t, cs, ko_ap, pps, tmp,
                            post_j=None):
                """One 512-column projection block with half-RoPE epilogue.

                cs = (cosA, cosB, sinA, sinB) quarter-split tiles so the
                WAR on each frees mid-block. ko_ap(j) -> (ap_lo, ap_hi):
                destination APs for output chunks j and j+8. post_j runs
                after the epilogue (e.g. to store the chunks)."""
                cosA, cosB, sinA, sinB = cs
                for j in range(8):
                    psA = pps.tile([P, 512], F32, tag="psA")
                    psB = pps.tile([P, 512], F32, tag="psB")
                    for di in range(DI):
                        nc.tensor.matmul(
                            psA, w_t[:, di, j * P:(j + 1) * P],
                            q_t[:, di, :],
                            start=(di == 0), stop=(di == DI - 1))
                    for di in range(DI):
                        nc.tensor.matmul(
                            psB, w_t[:, di, (j + 8) * P:(j + 9) * P],
                            q_t[:, di, :],
                            start=(di == 0), stop=(di == DI - 1))
                    ap_lo, ap_hi = ko_ap(j)
                    cj = cosA[:, j] if j < 4 else cosB[:, j - 4]
                    sj = sinA[:, j] if j < 4 else sinB[:, j - 4]
                    t1 = tmp.tile([P, 512], BF, tag="t1")
                    t2 = tmp.tile([P, 512], BF, tag="t2")
                    nc.vector.tensor_tensor(
                        t1, psA, cj, mybir.AluOpType.mult)
                    nc.vector.tensor_tensor(
                        t2, psB, sj, mybir.AluOpType.mult)
                    nc.vector.tensor_tensor(
                        ap_lo, t1, t2, mybir.AluOpType.subtract)
                    nc.vector.tensor_tensor(
                        t1, psA, sj, mybir.AluOpType.mult)
                    nc.vector.tensor_tensor(
                        t2, psB, cj, mybir.AluOpType.mult)
                    nc.vector.tensor_tensor(
                        ap_hi, t1, t2, mybir.AluOpType.add)
                    if post_j is not None:
                        post_j(j, ap_lo, ap_hi)

            # q3p sits at the bottom of the allocation stack: it carries the
            # last Q-projection pair (attention's first slot) across the
            # phase boundary, avoiding a serialized DRAM roundtrip there.
            q3p = top.enter_context(tc.tile_pool(name="q3p", bufs=1))
            q3_t = q3p.tile([P, DI, QB], BF, tag="q3")

            # --------- unified projection phase: K+V fused, then Q ---------
            with ExitStack() as pstk:
                qio = pstk.enter_context(tc.tile_pool(name="qio", bufs=2))
                csio = pstk.enter_context(tc.tile_pool(name="csio", bufs=1))
                kcp = pstk.enter_context(tc.tile_pool(name="kc", bufs=6))
                vop = pstk.enter_context(tc.tile_pool(name="vo", bufs=1))
                tmp = pstk.enter_context(tc.tile_pool(name="tmp", bufs=1))
                pps = pstk.enter_context(
                    tc.tile_pool(name="pps", bufs=2, space="PSUM"))
                vps = pstk.enter_context(
                    tc.tile_pool(name="vps", bufs=4, space="PSUM"))
                # weight pools on top of the stack so Wk's region can be
                # popped and re-pushed as Wq mid-phase
                w2_cm = tc.tile_pool(name="wv", bufs=1)
                w2p = w2_cm.__enter__()
                w1_cm = tc.tile_pool(name="wk", bufs=1)
                w1p = w1_cm.__enter__()

                wk_t = w1p.tile([P, DI, D], BF, tag="WK")
                wv_t = w2p.tile([P, DI, D], BF, tag="WV")
                wq_t = None
                for kb in range(4):
                    sl = slice(kb * 512, (kb + 1) * 512)
                    q_t = qio.tile([P, DI, 512], BF, tag="qin")
                    cs = (csio.tile([P, 4, 512], BF, tag="cosA", name="cosA"),
                          csio.tile([P, 4, 512], BF, tag="cosB", name="cosB"),
                          csio.tile([P, 4, 512], BF, tag="sinA", name="sinA"),
                          csio.tile([P, 4, 512], BF, tag="sinB", name="sinB"))
                    if kb == 0:
                        # startup: split q0+Wk chunk-interleaved across the
                        # sync and scalar queues; cos/sin go to gpsimd
                        qv = qT_keys[:, sl].rearrange("(c p) f -> p c f", p=P)
                        wkv = Wk[:, :].rearrange("(c p) f -> p c f", p=P)
                        for c in range(DI):
                            e = nc.sync if c % 2 == 0 else nc.scalar
                            e.dma_start(q_t[:, c], qv[:, c])
                            e.dma_start(wk_t[:, c], wkv[:, c])
                        _dma_in(nc, cs[0], cosK[0:HALF // 2, sl], 4,
                                eng=nc.gpsimd)
                        _dma_in(nc, cs[1], cosK[HALF // 2:HALF, sl], 4,
                                eng=nc.gpsimd)
                        _dma_in(nc, cs[2], sinK[0:HALF // 2, sl], 4,
                                eng=nc.gpsimd)
                        _dma_in(nc, cs[3], sinK[HALF // 2:HALF, sl], 4,
                                eng=nc.gpsimd)
                    else:
                        _dma_in(nc, q_t, qT_keys[:, sl], DI, eng=nc.sync)
                        _dma_in(nc, cs[0], cosK[0:HALF // 2, sl], 4,
                                eng=nc.sync)
                        _dma_in(nc, cs[1], cosK[HALF // 2:HALF, sl], 4,
                                eng=nc.sync)
                        _dma_in(nc, cs[2], sinK[0:HALF // 2, sl], 4,
                                eng=nc.sync)
                        _dma_in(nc, cs[3], sinK[HALF // 2:HALF, sl], 4,
                                eng=nc.sync)
                    kv_out = kvK[kb][:].rearrange(
                        "(d s) -> d s", s=KB).rearrange(
                        "(c p) f -> p c f", p=P)

                    def _koap(j, kcp=kcp):
                        lo = kcp.tile([P, 512], BF, tag="koc")
                        hi = kcp.tile([P, 512], BF, tag="koc")
                        return lo, hi

                    def _kstore(j, lo, hi, kv_out=kv_out, kb=kb):
                        nc.gpsimd.dma_start(kv_out[:, j], lo)
                        nc.gpsimd.dma_start(kv_out[:, j + 8], hi)
                        if kb == 0 and j == 0:
                            _dma_in(nc, wv_t, Wv, DI, eng=nc.gpsimd)

                    _rope_block(wk_t, q_t, cs, _koap, pps, tmp,
                                post_j=_kstore)
                    if kb == 3:
                        # free Wk's region and stream Wq into it, so it
                        # lands during block 3's V part; split across three
                        # queues (scalar only carries vo stores here)
                        w1_cm.__exit__(None, None, None)
                        w1_cm = tc.tile_pool(name="wq", bufs=1)
                        w1p = w1_cm.__enter__()
                        wq_t = w1p.tile([P, DI, D], BF, tag="WQ")
                        wqv = Wq[:, :].rearrange("(c p) f -> p c f", p=P)
                        wengs = [nc.sync, nc.gpsimd, nc.scalar]
                        for c in range(DI):
                            wengs[c % 3].dma_start(wq_t[:, c], wqv[:, c])
                        # hoist Q pair 3's inputs so they stream during
                        # block 3's V part
                        q_sb0 = qio.tile([P, DI, 512], BF, tag="qin")
                        _dma_in(nc, q_sb0, qT_own[:, 0:512], DI, eng=nc.sync)
                        cs_sb0 = (
                            csio.tile([P, 4, 512], BF, tag="cosA",
                                      name="cosA0"),
                            csio.tile([P, 4, 512], BF, tag="cosB",
                                      name="cosB0"),
                            csio.tile([P, 4, 512], BF, tag="sinA",
                                      name="sinA0"),
                            csio.tile([P, 4, 512], BF, tag="sinB",
                                      name="sinB0"))
                        _dma_in(nc, cs_sb0[0], cosO[0:HALF // 2, 0:512], 4,
                                eng=nc.sync)
                        _dma_in(nc, cs_sb0[1], cosO[HALF // 2:HALF, 0:512],
                                4, eng=nc.sync)
                        _dma_in(nc, cs_sb0[2], sinO[0:HALF // 2, 0:512], 4,
                                eng=nc.sync)
                        _dma_in(nc, cs_sb0[3], sinO[HALF // 2:HALF, 0:512],
                                4, eng=nc.sync)
                    # K AllGather fires as soon as the block's stores land
                    _ag(kvK[kb], gK[kb])
                    # V part reuses the same q tile
                    vv = kvV[kb][:].rearrange("(s d) -> s d", d=D)
                    for ss in range(4):
                        vo = vop.tile([P, D], BF, tag="vo")
                        for dob in range(4):
                            ps = vps.tile([P, 512], F32, tag="vps")
                            for di in range(DI):
                                nc.tensor.matmul(
                                    ps, q_t[:, di, ss * P:(ss + 1) * P],
                                    wv_t[:, di, dob * 512:(dob + 1) * 512],
                                    start=(di == 0), stop=(di == DI - 1))
                            nc.scalar.copy(
                                vo[:, dob * 512:(dob + 1) * 512], ps)
                        nc.scalar.dma_start(vv[ss * P:(ss + 1) * P, :], vo)
                    _ag(kvV[kb], gV[kb])

                # ---------------- Q projection (own query pairs) ----------
                # Blocks 0-2 stream to DRAM chunk-by-chunk; block 3 writes
                # into the resident q3 tile.
                for sb in range(NSLOT):
                    sl = slice(sb * 512, (sb + 1) * 512)
                    if sb == 0:
                        q_t = q_sb0
                        cs = cs_sb0
                    else:
                        q_t = qio.tile([P, DI, 512], BF, tag="qin")
                        _dma_in(nc, q_t, qT_own[:, sl], DI, eng=nc.sync)
                        cs = (csio.tile([P, 4, 512], BF, tag="cosA",
                                        name="cosA"),
                              csio.tile([P, 4, 512], BF, tag="cosB",
                                        name="cosB"),
                              csio.tile([P, 4, 512], BF, tag="sinA",
                                        name="sinA"),
                              csio.tile([P, 4, 512], BF, tag="sinB",
                                        name="sinB"))
                        _dma_in(nc, cs[0], cosO[0:HALF // 2, sl], 4,
                                eng=nc.sync)
                        _dma_in(nc, cs[1], cosO[HALF // 2:HALF, sl], 4,
                                eng=nc.sync)
                        _dma_in(nc, cs[2], sinO[0:HALF // 2, sl], 4,
                                eng=nc.sync)
                        _dma_in(nc, cs[3], sinO[HALF // 2:HALF, sl], 4,
                                eng=nc.sync)
                    if sb == 3:
                        def _koap_q(j):
                            return q3_t[:, j], q3_t[:, j + 8]
                        _rope_block(wq_t, q_t, cs, _koap_q,
                                    pps, tmp)
                    else:
                        qd_out = QT_d[:, sb * QB:(sb + 1) * QB].rearrange(
                            "(c p) f -> p c f", p=P)

                        def _koap_q(j, kcp=kcp):
                            lo = kcp.tile([P, 512], BF, tag="koc")
                            hi = kcp.tile([P, 512], BF, tag="koc")
                            return lo, hi

                        def _qstore(j, lo, hi, qd_out=qd_out):
                            nc.gpsimd.dma_start(qd_out[:, j], lo)
                            nc.gpsimd.dma_start(qd_out[:, j + 8], hi)

                        _rope_block(wq_t, q_t, cs, _koap_q,
                                    pps, tmp, post_j=_qstore)
                w1_cm.__exit__(None, None, None)   # wq
                w2_cm.__exit__(None, None, None)   # wv

            # ---- attention phase (PV output IS the final output) ---------
            # Pool order is load-bearing: pools whose first access happens
            # only after the projection phase drains (qslot, fop, acc,
            # small, mio) sit at the bottom, over SBUF that stays live to
            # the end of Q-proj; kio and vio then land inside the region
            # freed by Wv at the end of the V part, so the first K/V tiles
            # stream in during the whole Q projection.
            with ExitStack() as stk:
                qslot = stk.enter_context(tc.tile_pool(name="qslot", bufs=2))
                fop = stk.enter_context(tc.tile_pool(name="fop", bufs=1))
                accpool = stk.enter_context(tc.tile_pool(name="acc", bufs=2))
                smallp = stk.enter_context(tc.tile_pool(name="small",
                                                        bufs=1))
                mio = stk.enter_context(tc.tile_pool(name="mio", bufs=2))
                kio = stk.enter_context(tc.tile_pool(name="kio", bufs=2))
                vio = stk.enter_context(tc.tile_pool(name="vio", bufs=2))
                ptpool = stk.enter_context(tc.tile_pool(name="pt", bufs=2))
                otqp = stk.enter_context(tc.tile_pool(name="otq", bufs=2))
                stps = stk.enter_context(
                    tc.tile_pool(name="stps", bufs=2, space="PSUM"))
                pvps = stk.enter_context(
                    tc.tile_pool(name="pvps", bufs=3, space="PSUM"))
                lps = stk.enter_context(
                    tc.tile_pool(name="lps", bufs=1, space="PSUM"))
                ones_t = smallp.tile([P, 1], F32, tag="ones")
                nc.vector.memset(ones_t, 1.0)
                ones_bf = smallp.tile([P, 1], BF, tag="onesb")
                nc.vector.memset(ones_bf, 1.0)
                q_next = None
                for p in range(NSLOT):
                    t = TSTEPS[p]
                    q_t = q3_t if p == 0 else q_next
                    # otq accumulates in [query, d] orientation: partitions
                    # are the 128 queries of sub-chunk qc, free dim is D
                    otq = otqp.tile([P, 4, D], BF, tag="otq")
                    # masks for the two boundary steps of this pair (pair 0
                    # loads on scalar so the gpsimd queue reaches the first
                    # V tile's DMA without waiting on the mask region WAR)
                    meng = nc.scalar if p == 0 else nc.gpsimd
                    mF_t = mio.tile([P, 4, QB], BF, tag="mF")
                    _dma_in(nc, mF_t, masksF[p], 4, eng=meng)
                    mL_t = mio.tile([P, 4, QH], BF, tag="mL")
                    _dma_in(nc, mL_t, masksL[p], 4, eng=meng)
                    acc = accpool.tile([P, QB], F32, tag="acc")
                    for s in range(t):
                        lastw = (s == t - 1)
                        W = QH if lastw else QB
                        ag, idx = AGIDX[s]
                        # the first K/V tiles land in SBUF freed at the end
                        # of the V part, so these DMAs run during Q-proj
                        kt = kio.tile([P, DI, KB], BF, tag="kt")
                        _dma_in(nc, kt,
                                gK[ag][idx].rearrange("(d s) -> d s", s=KB),
                                DI,
                                eng=nc.scalar if (p == 0 and s == 0)
                                else nc.sync)
                        vt = vio.tile([P, 4, D], BF, tag="vt")
                        _dma_in(nc, vt,
                                gV[ag][idx].rearrange("(s d) -> s d", d=D),
                                4, eng=nc.gpsimd)
                        if s == 1 and p < NSLOT - 1:
                            # prefetch the next pair's Q block early
                            q_next = qslot.tile([P, DI, QB], BF, tag="qslot")
                            _dma_in(nc, q_next,
                                    QT_d[:, (2 - p) * QB:(3 - p) * QB], DI,
                                    eng=nc.scalar)
                        pt = ptpool.tile([P, 4, QB], BF, tag="pt")
                        for kc in range(4):
                            st = stps.tile([P, QB], F32, tag="st")
                            for di in range(DI):
                                nc.tensor.matmul(
                                    st[:, 0:W],
                                    kt[:, di, kc * P:(kc + 1) * P],
                                    q_t[:, di, 0:W],
                                    start=(di == 0), stop=(di == DI - 1))
                            if s == t - 2:
                                nc.vector.tensor_add(
                                    st, st, mF_t[:, kc])
                            if lastw:
                                nc.vector.tensor_add(
                                    st[:, 0:W], st[:, 0:W], mL_t[:, kc])
                            nc.scalar.activation(
                                pt[:, kc, 0:W], st[:, 0:W],
                                mybir.ActivationFunctionType.Exp)
                            # softmax denominator: accumulate on gpsimd
                            # through step t-2, reduce with one fp32
                            # ones-matmul, fold the narrow last step in
                            # with bf16 ones-matmuls
                            if s < t - 1:
                                if s == 0 and kc == 0:
                                    nc.gpsimd.tensor_copy(acc, pt[:, kc])
                                else:
                                    nc.gpsimd.tensor_tensor(
                                        acc, acc, pt[:, kc],
                                        mybir.AluOpType.add)
                        if s == t - 2:
                            l_ps = lps.tile([1, QB], F32, tag="lp")
                            nc.tensor.matmul(l_ps, ones_t, acc,
                                             start=True, stop=True)
                            l_sb = smallp.tile([1, QB], F32, tag="lsb",
                                               bufs=2)
                            nc.scalar.copy(l_sb, l_ps)
                        elif lastw:
                            l2_ps = lps.tile([1, QH], F32, tag="lp2")
                            for kc in range(4):
                                nc.tensor.matmul(
                                    l2_ps, ones_bf, pt[:, kc, 0:QH],
                                    start=(kc == 0), stop=(kc == 3))
                            l2_sb = smallp.tile([1, QH], F32, tag="lsb2",
                                                bufs=2)
                            nc.scalar.copy(l2_sb, l2_ps)
                            nc.vector.tensor_tensor(
                                l_sb[:, 0:QH], l_sb[:, 0:QH], l2_sb,
                                mybir.AluOpType.add)
                            nc.sync.dma_start(l_d[p:p + 1, :], l_sb)
                            lcols = smallp.tile([P, NSLOT], F32,
                                                tag="lcols", bufs=2)
                            nc.sync.dma_start(
                                lcols, l_d[p].rearrange("(qs p) -> p qs",
                                                        p=P))
                            inv_l = smallp.tile([P, NSLOT], F32,
                                                tag="invl", bufs=2)
                            nc.vector.reciprocal(inv_l, lcols)
                        # PV with pt stationary: pv[q, d] accumulates the
                        # (unnormalized) final output directly
                        nqc = W // P
                        for qc in range(nqc):
                            for dh in range(4):
                                pv = pvps.tile([P, 512], F32, tag="pv")
                                for kc in range(4):
                                    nc.tensor.matmul(
                                        pv,
                                        pt[:, kc, qc * P:(qc + 1) * P],
                                        vt[:, kc, dh * 512:(dh + 1) * 512],
                                        start=(kc == 0), stop=(kc == 3))
                                dsl = slice(dh * 512, (dh + 1) * 512)
                                if s == 0:
                                    nc.vector.tensor_copy(
                                        otq[:, qc, dsl], pv)
                                else:
                                    nc.vector.tensor_add(
                                        otq[:, qc, dsl], otq[:, qc, dsl],
                                        pv)
                    # epilogue: scale by the softmax denominator and store;
                    # overlaps the next pair's score matmuls
                    for qc in range(4):
                        fo = fop.tile([P, D], F32, tag="fo")
                        nc.scalar.mul(fo, otq[:, qc], inv_l[:, qc:qc + 1])
                        nc.scalar.dma_start(
                            out[p * QB + qc * P:p * QB + (qc + 1) * P, :],
                            fo)
    return nc


_NC_CACHE = None


def _get_nc():
    global _NC_CACHE
    if _NC_CACHE is None:
        _NC_CACHE = _build()
    return _NC_CACHE


def _host_prep(q, W_q, W_k, W_v, W_o):
    perm = np.concatenate([np.arange(0, D, 2), np.arange(1, D, 2)])
    scale = 1.0 / math.sqrt(D)
    Wq_p = np.ascontiguousarray((W_q * scale)[:, perm]).astype(bf16)
    Wk_p = np.ascontiguousarray(W_k[:, perm]).astype(bf16)
    # fold the output projection into the value projection:
    # (P @ (q Wv)) @ Wo == P @ (q @ (Wv Wo))
    Wvo_p = (W_v.astype(np.float32) @ W_o.astype(np.float32)).astype(bf16)
    inv_freq = 1.0 / (ROPE_BASE ** (np.arange(0, D, 2, dtype=np.float64) / D))
    ang = np.arange(S, dtype=np.float64)[:, None] * inv_freq[None, :]
    cosT = np.ascontiguousarray(np.cos(ang).T).astype(bf16)   # (HALF, S)
    sinT = np.ascontiguousarray(np.sin(ang).T).astype(bf16)
    return Wq_p, Wk_p, Wvo_p, cosT, sinT


def _make_masks(pairs):
    mF = np.zeros((NSLOT, KB, QB), dtype=np.float32)
    mL = np.zeros((NSLOT, KB, QH), dtype=np.float32)
    k = np.arange(KB)[:, None]
    qh = np.arange(QH)
    for p, (hA, hB) in enumerate(pairs):
        t = TSTEPS[p]
        gq = np.concatenate([hA * QH + qh, hB * QH + qh])[None, :]
        mF[p] = np.where((t - 2) * KB + k <= gq, 0.0, NEG)
        mL[p] = np.where((t - 1) * KB + k <= (hA * QH + qh)[None, :],
                         0.0, NEG)
    return mF.astype(bf16), mL.astype(bf16)


def run(inputs, trace=False):
    q = np.asarray(inputs["q"], dtype=np.float32)
    W_q = np.asarray(inputs["W_q"], dtype=np.float32)
    W_k = np.asarray(inputs["W_k"], dtype=np.float32)
    W_v = np.asarray(inputs["W_v"], dtype=np.float32)
    W_o = np.asarray(inputs["W_o"], dtype=np.float32)

    Wq_p, Wk_p, Wvo_p, cosT, sinT = _host_prep(q, W_q, W_k, W_v, W_o)

    in_maps = []
    core_pairs = []
    for c in range(8):
        b = c // 2
        pairs = PAIRS_EVEN if c % 2 == 0 else PAIRS_ODD
        keys = KEYS_EVEN if c % 2 == 0 else KEYS_ODD
        core_pairs.append((b, pairs))
        qTb = np.ascontiguousarray(q[b].T).astype(bf16)       # (D, S)
        # own query columns: projection blocks sb=0..3 hold pairs 3-sb,
        # each pair contributing its A then B 256-column sub-slots
        own_cols = np.concatenate(
            [np.arange(h * QH, (h + 1) * QH)
             for sb in range(NSLOT)
             for h in pairs[NSLOT - 1 - sb]])
        key_cols = np.concatenate(
            [np.arange(blk * KB, (blk + 1) * KB) for blk in keys])
        mF, mL = _make_masks(pairs)
        in_maps.append({
            "qT_own": np.ascontiguousarray(qTb[:, own_cols]),
            "qT_keys": np.ascontiguousarray(qTb[:, key_cols]),
            "Wq": Wq_p, "Wk": Wk_p, "Wv": Wvo_p,
            "cosO": np.ascontiguousarray(cosT[:, own_cols]),
            "sinO": np.ascontiguousarray(sinT[:, own_cols]),
            "cosK": np.ascontiguousarray(cosT[:, key_cols]),
            "sinK": np.ascontiguousarray(sinT[:, key_cols]),
            "masksF": mF,
            "masksL": mL,
        })

    nc = _get_nc()
    res = run_bass_kernel_spmd(nc, in_maps, core_ids=list(range(8)),
                               trace=trace)

    out = np.zeros((B, S, D), dtype=np.float32)
    for c, (b, pairs) in enumerate(core_pairs):
        o = res.results[c]["out"]
        for p, (hA, hB) in enumerate(pairs):
            out[b, hA * QH:(hA + 1) * QH] = o[p * QB:p * QB + QH]
            out[b, hB * QH:(hB + 1) * QH] = o[p * QB + QH:(p + 1) * QB]
    return out, res


def kernel(**inputs):
    return run(inputs, trace=False)[0]
